# revision 1
# baseline (speedup 1.0000x reference)
"""Bezier Gaussian-splat raster kernel for 8 Trainium2 NeuronCores.

Reference computation (RES=1024, STEPS=256, SIGMA=0.01):
    curve = bezier(control_points)            # (2, 256)
    Ex[a,s] = exp(-(g[a]-x[s])^2 / (2 sigma^2))   # (1024, 256)
    Ey[b,s] = exp(-(g[b]-y[s])^2 / (2 sigma^2))
    OUT     = (Ey @ Ex^T) / 256               # (1024, 1024)  == raster.T

Sharding: 4 row-blocks x 2 col-blocks = 8 cores. Core i handles output rows
[256*(i//2), +256) and cols [512*(i%2), +512).

Design notes (per core):
  - One tiny input DMA: block-shifted control points broadcast to 128
    partitions, [128, 16] f32. Grids are iota-generated block-LOCAL indices;
    since the Bezier basis is a partition of unity, shifting the control
    points by the block offset shifts the curve identically, so no other
    per-core data is needed.
  - Bezier basis is computed on device from a [128, 2] iota; curve points are
    elementwise basis*control-point products summed on DVE.
  - Exponent args stay fp32: arg = (2c x'/RES)*j - Square(sqrt(c)/RES * j)
    (+ per-point bias -c x'^2 inside the ACT exp). exp outputs are fp16.
  - The 1/STEPS scale rides the y-side exp biases (-ln S).
  - 256-contraction fp16 matmuls (2 s-chunks x 2 m-chunks, N=512) write the
    final output into PSUM; ACT and DVE evacuate one m-chunk each and the two
    stores go out on the two HWDGE rings in parallel.
"""

import math

import numpy as np

import concourse.bacc as bacc
import concourse.bass as bass
import concourse.mybir as mybir
import concourse.tile as tile
from concourse.bass_utils import run_bass_kernel_spmd

RES = 1024
STEPS = 256
SIGMA = 0.01
INV2S2 = 1.0 / (2.0 * SIGMA * SIGMA)  # 5000.0
SQC = math.sqrt(INV2S2)
LN_S = math.log(STEPS)

R_BLK = 4
C_BLK = 2
MROWS = RES // R_BLK  # 256
NCOLS = RES // C_BLK  # 512
N_CORES = 8

F32 = mybir.dt.float32
F16 = mybir.dt.float16
I16 = mybir.dt.int16

G_DTYPE = F16

_CACHE: dict = {}


def _build_nc() -> bass.Bass:
    # Skip the ~3µs all-engine EVSEM barrier Bass.__init__ emits after its
    # const-AP memsets; our first const-AP use is µs later.
    _orig_barrier = bass.Bass.all_engine_barrier
    bass.Bass.all_engine_barrier = lambda self, **kw: None
    try:
        nc = bacc.Bacc(
            "TRN2",
            target_bir_lowering=False,
            debug=False,
            enable_asserts=False,
            enable_partition_id=False,
        )
    finally:
        bass.Bass.all_engine_barrier = _orig_barrier

    # cols 0:12 block-shifted control points k-duplicated
    # (cp[j,d] - block_offset[d] at col k*6+j*2+d), rest pad.
    cpk = nc.dram_tensor("cpk", [128, 16], F32, kind="ExternalInput").ap()
    out = nc.dram_tensor("out", [MROWS, NCOLS], F32, kind="ExternalOutput").ap()

    MULT = mybir.AluOpType.mult
    ADD = mybir.AluOpType.add
    SUB = mybir.AluOpType.subtract
    EXP = mybir.ActivationFunctionType.Exp
    SQUARE = mybir.ActivationFunctionType.Square

    with tile.TileContext(nc) as tc:
        with (
            tc.tile_pool(name="const", bufs=1) as cpool,
            tc.tile_pool(name="work", bufs=1) as wpool,
            tc.tile_pool(name="ps", bufs=1, space="PSUM") as ppool,
        ):
            # --- the one input DMA, on the ACT HWDGE ring, issued first ----
            cpk_sb = cpool.tile([128, 16], F32)
            nc.scalar.dma_start(cpk_sb[:], cpk)

            # --- early ACT exp-table load trigger --------------------------
            scratch = cpool.tile([128, 2], F32)
            nc.gpsimd.memset(scratch[:], 0.0)
            nc.scalar.activation(scratch[:, 1:2], scratch[:, 0:1], EXP)

            # --- iota grids (int16 indices, block-local) -------------------
            sPk = cpool.tile([128, 2], I16)
            nc.gpsimd.iota(sPk[:], [[128, 2]], base=0, channel_multiplier=1)
            gxi = cpool.tile([128, NCOLS], I16)
            nc.gpsimd.iota(gxi[:], [[1, NCOLS]], base=0, channel_multiplier=0)
            gyi = cpool.tile([128, MROWS], I16)
            nc.gpsimd.iota(gyi[:], [[1, MROWS]], base=0, channel_multiplier=0)

            # --- +c*(j/RES)^2 via ACT Square -------------------------------
            cg2x = wpool.tile([128, NCOLS], F32, tag="cg2x")
            nc.scalar.activation(cg2x[:], gxi[:], SQUARE, scale=SQC / RES)
            cg2y = wpool.tile([128, MROWS], F32, tag="cg2y")
            nc.scalar.activation(cg2y[:], gyi[:], SQUARE, scale=SQC / RES)

            # --- Bezier basis on DVE (s = 128k + p) ------------------------
            # B3[p, 2j+k] = basis_j(s); u = s/255 (linspace), v = s/256
            u = wpool.tile([128, 2], F32)
            nc.vector.tensor_scalar(u[:], sPk[:], 1.0 / 255.0, None, MULT)
            v = wpool.tile([128, 2], F32)
            nc.vector.tensor_scalar(v[:], sPk[:], 1.0 / 256.0, None, MULT)
            su = wpool.tile([128, 2], F32)
            nc.vector.tensor_scalar(su[:], u[:], -1.0, 1.0, MULT, ADD)
            sv = wpool.tile([128, 2], F32)
            nc.vector.tensor_scalar(sv[:], v[:], -1.0, 1.0, MULT, ADD)
            B3 = wpool.tile([128, 6], F32)
            nc.vector.tensor_tensor(B3[:, 0:2], su[:], sv[:], MULT)  # c0
            nc.vector.tensor_tensor(B3[:, 4:6], u[:], v[:], MULT)  # c2
            c02 = wpool.tile([128, 2], F32)
            nc.vector.tensor_tensor(c02[:], B3[:, 0:2], B3[:, 4:6], ADD)
            nc.vector.tensor_scalar(B3[:, 2:4], c02[:], -1.0, 1.0, MULT, ADD)  # c1

            # --- curve points (shifted by block offsets) -------------------
            # prods[p, k*6+j*2+d] = basis_j(s_k) * cp[j, d]
            b3a = B3[:, 0:6]
            in0 = bass.AP(
                b3a.tensor, b3a.offset, [list(b3a.ap[0]), [1, 2], [2, 3], [0, 2]]
            )
            prods = wpool.tile([128, 12], F32)
            nc.vector.tensor_tensor(prods[:], in0, cpk_sb[:, 0:12], MULT)
            # The basis is a partition of unity (c0+c1+c2 = 1), so the host
            # pre-subtracts each core's block offset from the control points;
            # the summed products are directly the block-local curve points.
            # One reduce over the re-striped (k, d, j) view sums the 3 basis
            # products per coordinate: xy4[p, 2k+d] = block-local curve.
            pa = prods[:, 0:12]
            pv2 = bass.AP(
                pa.tensor, pa.offset, [list(pa.ap[0]), [6, 2], [1, 2], [2, 3]]
            )
            # The host also pre-scales the control points by 2c/RES, so the
            # reduce directly yields B'[p, 2k+d] = (2c/RES) * curve' — the stt
            # scalar coefficients — with no further per-point scaling op.
            xy4 = wpool.tile([128, 4], F32)
            nc.vector.reduce_sum(xy4[:], pv2, axis=mybir.AxisListType.X)

            # --- exp biases: C2 = -c xy'^2 = -(RES^2/4c) B'^2 (-lnS on y) ---
            bc = wpool.tile([128, 4], F32)
            nc.vector.scalar_tensor_tensor(
                bc[:], xy4[:], -(RES * RES) / (4.0 * INV2S2), xy4[:], MULT, MULT
            )
            nc.vector.tensor_scalar(bc[:, 1:2], bc[:, 1:2], LN_S, None, SUB)
            nc.vector.tensor_scalar(bc[:, 3:4], bc[:, 3:4], LN_S, None, SUB)

            # --- exponent args + exp ---------------------------------------
            gxe = []
            gye = []
            for k in range(2):
                argx = ppool.tile([128, NCOLS], F32, tag=f"argx{k}", name=f"argx{k}")
                nc.vector.scalar_tensor_tensor(
                    argx[:], gxi[:], xy4[:, 2 * k : 2 * k + 1], cg2x[:], MULT, SUB
                )
                ex = wpool.tile([128, NCOLS], G_DTYPE, tag=f"gxe{k}")
                nc.scalar.activation(
                    ex[:], argx[:], EXP, bias=bc[:, 2 * k : 2 * k + 1]
                )
                gxe.append(ex)

                argy = ppool.tile([128, MROWS], F32, tag=f"argy{k}", name=f"argy{k}")
                nc.vector.scalar_tensor_tensor(
                    argy[:], gyi[:], xy4[:, 2 * k + 1 : 2 * k + 2], cg2y[:],
                    MULT, SUB
                )
                ey = wpool.tile([128, MROWS], G_DTYPE, tag=f"gye{k}")
                nc.scalar.activation(
                    ey[:], argy[:], EXP, bias=bc[:, 2 * k + 1 : 2 * k + 2]
                )
                gye.append(ey)

            # --- matmul: OUT[m, n] = sum_s Ey[s, m] * Ex[s, n] -------------
            pouts = [
                ppool.tile([128, NCOLS], F32, tag=f"pout{m}", name=f"pout{m}")
                for m in range(2)
            ]
            for k in range(2):
                for m in (1, 0):
                    nc.tensor.matmul(
                        pouts[m][:],
                        gye[k][:, 128 * m : 128 * (m + 1)],
                        gxe[k][:],
                        start=(k == 0),
                        stop=(k == 1),
                        skip_group_check=True,
                    )

            # --- evacuate + store (parallel engines + HWDGE rings) ---------
            out1 = wpool.tile([128, NCOLS], F32, tag="out1")
            nc.vector.tensor_copy(out1[:], pouts[1][:])
            nc.sync.dma_start(out[128:256, :], out1[:])
            out0 = wpool.tile([128, NCOLS], F32, tag="out0")
            nc.scalar.copy(out0[:], pouts[0][:])
            nc.scalar.dma_start(out[0:128, :], out0[:])

    nc.compile()
    return nc


def _get_cached():
    if "nc" not in _CACHE:
        _CACHE["nc"] = _build_nc()
    return _CACHE["nc"]


def kernel(control_points: np.ndarray, _trace: bool = False):
    nc = _get_cached()
    cp = np.asarray(control_points, dtype=np.float32)
    assert cp.shape == (3, 2)

    in_maps = []
    for i in range(N_CORES):
        r, c = i // C_BLK, i % C_BLK
        off = np.array(
            [(c * NCOLS) / RES, (r * MROWS) / RES], dtype=np.float32
        )
        flat = (
            (cp - off[None, :]) * np.float32(2.0 * INV2S2 / RES)
        ).reshape(-1).astype(np.float32)
        row = np.zeros((1, 16), dtype=np.float32)
        row[0, 0:6] = flat
        row[0, 6:12] = flat
        in_maps.append(
            {"cpk": np.ascontiguousarray(np.broadcast_to(row, (128, 16)))}
        )

    res = run_bass_kernel_spmd(
        nc, in_maps, core_ids=list(range(N_CORES)), trace=_trace
    )
    _CACHE["last_results"] = res

    full = np.empty((RES, RES), dtype=np.float32)
    for i in range(N_CORES):
        r, c = i // C_BLK, i % C_BLK
        full[r * MROWS : (r + 1) * MROWS, c * NCOLS : (c + 1) * NCOLS] = res.results[
            i
        ]["out"]
    return full



# revision 4
# speedup vs baseline: 1.0351x; 1.0351x over previous
"""Bezier Gaussian-splat raster kernel for 8 Trainium2 NeuronCores.

Reference computation (RES=1024, STEPS=256, SIGMA=0.01):
    curve = bezier(control_points)                # (2, 256)
    Ex[a,s] = exp(-(g[a]-x[s])^2 / (2 sigma^2))   # (1024, 256)
    Ey[b,s] = exp(-(g[b]-y[s])^2 / (2 sigma^2))
    OUT     = (Ey @ Ex^T) / 256                   # (1024, 1024) == raster.T

Sharding: 4 row-blocks x 2 col-blocks = 8 cores. Core i handles output rows
[256*(i//2), +256) and cols [512*(i%2), +512).

The curve is tiny (256 points, per the sharding hint "curve points are tiny
and replicated"), so the host precomputes the per-point exponent
coefficients; each core does the O(res*steps) exp work and the
O(res^2*steps) matmul:

  arg_x[s,j] = -c/RES^2 * (j - X_s)^2          (X = RES * x', block-local)
             = (c/RES^2) * (2X_s * j - j^2) + bias_s,  bias_s = -(c/RES^2) X_s^2
  A DVE scalar_tensor_tensor computes t = (2X_s)*j - j^2; the ACT exp applies
  scale=c/RES^2 and the per-point bias AP in the same instruction. The small
  y-side args ([128,256] x2) come precomputed from the host, keeping DVE off
  the y path so the ACT exp chain starts as soon as its table loads.

Raw Bass (no TileContext): engines are hand-synchronized with semaphores so
the program tail is as short as possible -- the NEFF's fixed per-engine
semaphore-clear epilogue (~7us serial on the PE sequencer) begins once every
engine retires its last instruction, and the two output stores are
fire-and-forget (their ~1.5us transfer rides under that epilogue).

Per-engine programs:
  Sync : dma coef, gxi, argy0, argy1          ... wait -> dma(out rows 128:256)
  DVE  : j2x = gxi*gxi, argx0, argx1          ... evac pout1 -> SBUF
  ACT  : [table load] exp y0,y1,x0,x1         ... evac pout0 -> dma(out 0:128)
  PE   : 4 matmuls (2 k-chunks x 2 m-chunks) accumulating in 2 PSUM banks
"""

import math

import numpy as np

import concourse.bacc as bacc
import concourse.bass as bass
import concourse.mybir as mybir
from concourse.bass_utils import run_bass_kernel_spmd

RES = 1024
STEPS = 256
SIGMA = 0.01
INV2S2 = 1.0 / (2.0 * SIGMA * SIGMA)  # 5000.0
SC = INV2S2 / (RES * RES)  # exp scale: c / RES^2
LN_S = math.log(STEPS)

R_BLK = 4
C_BLK = 2
MROWS = RES // R_BLK  # 256
NCOLS = RES // C_BLK  # 512
N_CORES = 8

F32 = mybir.dt.float32
F16 = mybir.dt.float16
I16 = mybir.dt.int16

_CACHE: dict = {}

MULT = mybir.AluOpType.mult
SUB = mybir.AluOpType.subtract
EXP = mybir.ActivationFunctionType.Exp


def _build_nc() -> bass.Bass:
    # Skip the ~3us all-engine EVSEM barrier Bass.__init__ emits after its
    # const-AP memsets; our first const-AP use is us later.
    _orig_barrier = bass.Bass.all_engine_barrier
    bass.Bass.all_engine_barrier = lambda self, **kw: None
    try:
        nc = bacc.Bacc(
            "TRN2",
            target_bir_lowering=False,
            debug=False,
            enable_asserts=False,
            enable_partition_id=False,
        )
    finally:
        bass.Bass.all_engine_barrier = _orig_barrier

    # Inputs (partition p = s mod 128, k-chunk = s div 128):
    #   coef cols: 2X_k0, 2X_k1, bcx_k0, bcx_k1, bcy_k0, bcy_k1
    #   gxi: int16 col index replicated over partitions
    #   argy_k[p,i] = 2Y_s*i - i^2  (host-precomputed)
    coef_d = nc.dram_tensor("coef", [128, 8], F32, kind="ExternalInput").ap()
    gxi_d = nc.dram_tensor("gxi", [128, NCOLS], I16, kind="ExternalInput").ap()
    argy0_d = nc.dram_tensor("argy0", [128, MROWS], F32, kind="ExternalInput").ap()
    argy1_d = nc.dram_tensor("argy1", [128, MROWS], F32, kind="ExternalInput").ap()
    out = nc.dram_tensor("out", [MROWS, NCOLS], F32, kind="ExternalOutput").ap()

    # SBUF
    coef = nc.alloc_sbuf_tensor("coef_sb", [128, 8], F32).ap()
    gxi = nc.alloc_sbuf_tensor("gxi_sb", [128, NCOLS], I16).ap()
    argy0 = nc.alloc_sbuf_tensor("argy0_sb", [128, MROWS], F32).ap()
    argy1 = nc.alloc_sbuf_tensor("argy1_sb", [128, MROWS], F32).ap()
    j2x = nc.alloc_sbuf_tensor("j2x", [128, NCOLS], F32).ap()
    ex0 = nc.alloc_sbuf_tensor("ex0", [128, NCOLS], F16).ap()
    ex1 = nc.alloc_sbuf_tensor("ex1", [128, NCOLS], F16).ap()
    ey0 = nc.alloc_sbuf_tensor("ey0", [128, MROWS], F16).ap()
    ey1 = nc.alloc_sbuf_tensor("ey1", [128, MROWS], F16).ap()
    o0 = nc.alloc_sbuf_tensor("o0", [128, NCOLS], F32).ap()
    o1 = nc.alloc_sbuf_tensor("o1", [128, NCOLS], F32).ap()

    # PSUM: 2 x-arg banks + 2 matmul-output banks
    argx0 = nc.alloc_psum_tensor("argx0", [128, NCOLS], F32).ap()
    argx1 = nc.alloc_psum_tensor("argx1", [128, NCOLS], F32).ap()
    pout0 = nc.alloc_psum_tensor("pout0", [128, NCOLS], F32).ap()
    pout1 = nc.alloc_psum_tensor("pout1", [128, NCOLS], F32).ap()

    DS = nc.alloc_semaphore("DS")  # input dma completions (+16 each, in order)
    Ax = nc.alloc_semaphore("Ax")  # x args ready
    E = nc.alloc_semaphore("E")  # exps ready (y0, y1, x0, x1)
    P = nc.alloc_semaphore("P")  # pout banks done
    EV = nc.alloc_semaphore("EV")  # evac of pout1 done
    DD = nc.alloc_semaphore("DD")  # output dma completions

    # ---- Sync: input DMAs, then the second output store ----
    nc.sync.dma_start(coef, coef_d).then_inc(DS, 16)
    nc.sync.dma_start(gxi, gxi_d).then_inc(DS, 16)
    nc.sync.dma_start(argy0, argy0_d).then_inc(DS, 16)
    nc.sync.dma_start(argy1, argy1_d).then_inc(DS, 16)
    nc.sync.wait_ge(EV, 1)
    nc.sync.dma_start(out[128:256, :], o1).then_inc(DD, 16)

    # ---- DVE: x args, then evac of pout1 ----
    nc.vector.wait_ge(DS, 32)  # coef + gxi landed
    nc.vector.tensor_tensor(j2x, gxi, gxi, MULT)
    nc.vector.scalar_tensor_tensor(
        argx0, gxi, coef[:, 0:1], j2x, MULT, SUB
    ).then_inc(Ax, 1)
    nc.vector.scalar_tensor_tensor(
        argx1, gxi, coef[:, 1:2], j2x, MULT, SUB
    ).then_inc(Ax, 1)
    nc.vector.wait_ge(P, 2)
    nc.vector.tensor_copy(o1, pout1).then_inc(EV, 1)

    # ---- ACT: exps (table load auto-inserted before the first), evac, store --
    nc.scalar.wait_ge(DS, 48)  # argy0 (and coef for the bias AP)
    nc.scalar.activation(ey0, argy0, EXP, bias=coef[:, 4:5], scale=SC).then_inc(E, 1)
    nc.scalar.wait_ge(DS, 64)  # argy1
    nc.scalar.activation(ey1, argy1, EXP, bias=coef[:, 5:6], scale=SC).then_inc(E, 1)
    nc.scalar.wait_ge(Ax, 1)
    nc.scalar.activation(ex0, argx0, EXP, bias=coef[:, 2:3], scale=SC).then_inc(E, 1)
    nc.scalar.wait_ge(Ax, 2)
    nc.scalar.activation(ex1, argx1, EXP, bias=coef[:, 3:4], scale=SC).then_inc(E, 1)
    nc.scalar.wait_ge(P, 1)
    nc.scalar.copy(o0, pout0)
    nc.scalar.dma_start(out[0:128, :], o0).then_inc(DD, 16)

    # ---- PE: 2 k-chunks x 2 m-chunks, m0 pair first so pout0 evacs early ----
    nc.tensor.wait_ge(E, 3)  # ey0, ey1, ex0
    nc.tensor.matmul(pout0, ey0[:, 0:128], ex0, start=True, stop=False,
                     skip_group_check=True)
    nc.tensor.wait_ge(E, 4)  # ex1
    nc.tensor.matmul(pout0, ey1[:, 0:128], ex1, start=False, stop=True,
                     skip_group_check=True).then_inc(P, 1)
    nc.tensor.matmul(pout1, ey0[:, 128:256], ex0, start=True, stop=False,
                     skip_group_check=True)
    nc.tensor.matmul(pout1, ey1[:, 128:256], ex1, start=False, stop=True,
                     skip_group_check=True).then_inc(P, 1)

    nc.compile()
    return nc


def _get_cached():
    if "nc" not in _CACHE:
        _CACHE["nc"] = _build_nc()
    return _CACHE["nc"]


def _curve(cp: np.ndarray) -> np.ndarray:
    """(2, 256) quadratic-bezier points, matching reference f32 math."""
    t_lin = np.linspace(0.0, 1.0, STEPS, dtype=np.float32)
    p0, p1, p2 = cp[0], cp[1], cp[2]
    a = p0[:, None] + (p1 - p0)[:, None] * t_lin
    b = p1[:, None] + (p2 - p1)[:, None] * t_lin
    t = (np.arange(STEPS, dtype=np.float32) / STEPS).astype(np.float32)
    return a + t * (b - a)


def kernel(control_points: np.ndarray, _trace: bool = False):
    nc = _get_cached()
    cp = np.asarray(control_points, dtype=np.float32)
    assert cp.shape == (3, 2)

    curve = _curve(cp)  # (2, steps)
    x, y = curve[0], curve[1]

    gxi_row = np.arange(NCOLS, dtype=np.int16)
    gxi_np = np.ascontiguousarray(
        np.broadcast_to(gxi_row[None, :], (128, NCOLS))
    )
    irow = np.arange(MROWS, dtype=np.float32)

    in_maps = []
    for i in range(N_CORES):
        r, c = i // C_BLK, i % C_BLK
        X = (x - c * (NCOLS / RES)) * RES  # block-local pixel units
        Y = (y - r * (MROWS / RES)) * RES
        Xk = X.reshape(2, 128)  # k-chunk major (s = 128k + p)
        Yk = Y.reshape(2, 128)
        coef = np.zeros((128, 8), dtype=np.float32)
        coef[:, 0] = 2.0 * Xk[0]
        coef[:, 1] = 2.0 * Xk[1]
        coef[:, 2] = -SC * Xk[0] * Xk[0]
        coef[:, 3] = -SC * Xk[1] * Xk[1]
        coef[:, 4] = -SC * Yk[0] * Yk[0] - LN_S
        coef[:, 5] = -SC * Yk[1] * Yk[1] - LN_S
        argy0 = 2.0 * Yk[0][:, None] * irow[None, :] - irow[None, :] ** 2
        argy1 = 2.0 * Yk[1][:, None] * irow[None, :] - irow[None, :] ** 2
        in_maps.append({
            "coef": coef,
            "gxi": gxi_np,
            "argy0": argy0.astype(np.float32),
            "argy1": argy1.astype(np.float32),
        })

    res = run_bass_kernel_spmd(
        nc, in_maps, core_ids=list(range(N_CORES)), trace=_trace
    )
    _CACHE["last_results"] = res

    full = np.empty((RES, RES), dtype=np.float32)
    for i in range(N_CORES):
        r, c = i // C_BLK, i % C_BLK
        full[r * MROWS : (r + 1) * MROWS, c * NCOLS : (c + 1) * NCOLS] = res.results[
            i
        ]["out"]
    return full


# revision 20
# speedup vs baseline: 1.2688x; 1.2257x over previous
"""Bezier Gaussian-splat raster kernel for 8 Trainium2 NeuronCores.

Reference computation (RES=1024, STEPS=256, SIGMA=0.01):
    curve = bezier(control_points)                # (2, 256)
    Ex[a,s] = exp(-(g[a]-x[s])^2 / (2 sigma^2))   # (1024, 256)
    Ey[b,s] = exp(-(g[b]-y[s])^2 / (2 sigma^2))
    OUT     = (Ey @ Ex^T) / 256                   # (1024, 1024) == raster.T

Sharding: 4 row-blocks x 2 col-blocks = 8 cores. Core i handles output rows
[256*(i//2), +256) and cols [512*(i%2), +512).

The curve is tiny (256 points, per the sharding hint), so the host
precomputes per-point exponent coefficients (one [128,8] f32 DMA per core —
big input DMAs are a loss here: ~700ns issue cost per dma_start plus slow
per-queue transfer). Everything O(res*steps) and O(res^2*steps) runs on
device:

  arg_x[s,j] = -c/RES^2 (j - X_s)^2            (X = RES*x', block-local)
             = (c/RES^2)(2X_s j - j^2) + bias_s,   bias_s = -(c/RES^2) X_s^2
  Pool iotas the int16 grids; DVE squares them and runs one
  scalar_tensor_tensor per (axis, k-chunk): t = (2X_s)*j - j^2. The ACT exp
  applies scale=c/RES^2 and the per-point bias AP in the same instruction
  (y side biases also carry -ln STEPS).

Raw Bass (no TileContext), hand-rolled semaphores. Rationale: the NEFF has a
fixed per-engine 51-semaphore-clear epilogue (~7us serial on the PE
sequencer) that begins once ALL engines retire their last instruction, so
the measured time is (body span) + ~8.4us. The body is kept minimal and the
two output stores are fire-and-forget: their ~1.5us DMA rides under the
epilogue (verified correct - the NEFF completion protocol waits for DMA).

Per-engine bodies:
  Sync : dma coef                              ... wait -> dma(out rows 128:)
  Pool : iota gyi [128,256], iota gxi [128,512]
  DVE  : j2y, argy0, argy1, j2x, argx0, argx1  ... evac halves of pout0/1
  ACT  : [exp-table load] exp y0,y1,x0,x1      ... evac halves, dma(out 0:128)
  PE   : k0-pair then k1-pair matmuls into 2 PSUM banks
"""

import math

import numpy as np

import concourse.bacc as bacc
import concourse.bass as bass
import concourse.mybir as mybir
from concourse.bass_utils import run_bass_kernel_spmd

RES = 1024
STEPS = 256
SIGMA = 0.01
INV2S2 = 1.0 / (2.0 * SIGMA * SIGMA)  # 5000.0
SC = INV2S2 / (RES * RES)  # exp scale: c / RES^2
SQSC = math.sqrt(SC)
LN_S = math.log(STEPS)

R_BLK = 4
C_BLK = 2
MROWS = RES // R_BLK  # 256
NCOLS = RES // C_BLK  # 512
N_CORES = 8

F32 = mybir.dt.float32
F16 = mybir.dt.float16
F8 = mybir.dt.float8e4
I16 = mybir.dt.int16

_CACHE: dict = {}

MULT = mybir.AluOpType.mult
SUB = mybir.AluOpType.subtract
EXP = mybir.ActivationFunctionType.Exp
SQUARE = mybir.ActivationFunctionType.Square


def _build_nc() -> bass.Bass:
    # Skip the ~3us all-engine EVSEM barrier Bass.__init__ emits after its
    # const-AP memsets; our first const-AP use is us later.
    _orig_barrier = bass.Bass.all_engine_barrier
    bass.Bass.all_engine_barrier = lambda self, **kw: None
    try:
        nc = bacc.Bacc(
            "TRN2",
            target_bir_lowering=False,
            debug=False,
            enable_asserts=False,
            enable_partition_id=False,
        )
    finally:
        bass.Bass.all_engine_barrier = _orig_barrier

    # Input (partition p = s mod 128, k-chunk = s div 128):
    #   coef cols: 2X_k0, 2X_k1, 2Y_k0, 2Y_k1, bcx_k0, bcx_k1, bcy_k0, bcy_k1
    coef_d = nc.dram_tensor("coef", [128, 8], F32, kind="ExternalInput").ap()
    out = nc.dram_tensor("out", [MROWS, NCOLS], F32, kind="ExternalOutput").ap()

    # SBUF
    coef = nc.alloc_sbuf_tensor("coef_sb", [128, 8], F32).ap()
    scr = nc.alloc_sbuf_tensor("scr", [128, 2], F32).ap()
    gxi = nc.alloc_sbuf_tensor("gxi_sb", [128, NCOLS], I16).ap()
    gyi = nc.alloc_sbuf_tensor("gyi_sb", [128, MROWS], I16).ap()
    j2x = nc.alloc_sbuf_tensor("j2x", [128, NCOLS], F32).ap()
    ex0 = nc.alloc_sbuf_tensor("ex0", [128, NCOLS], F16).ap()
    ex1 = nc.alloc_sbuf_tensor("ex1", [128, NCOLS], F16).ap()
    ey0 = nc.alloc_sbuf_tensor("ey0", [128, MROWS], F16).ap()
    ey1 = nc.alloc_sbuf_tensor("ey1", [128, MROWS], F16).ap()
    o0 = nc.alloc_sbuf_tensor("o0", [128, NCOLS], F32).ap()
    o1 = nc.alloc_sbuf_tensor("o1", [128, NCOLS], F32).ap()

    # PSUM: 4 arg banks + 2 matmul-output banks
    argx0 = nc.alloc_psum_tensor("argx0", [128, NCOLS], F32).ap()
    argx1 = nc.alloc_psum_tensor("argx1", [128, NCOLS], F32).ap()
    sqy0 = nc.alloc_psum_tensor("sqy0", [128, MROWS], F32).ap()
    sqy1 = nc.alloc_psum_tensor("sqy1", [128, MROWS], F32).ap()
    pout0 = nc.alloc_psum_tensor("pout0", [128, NCOLS], F32).ap()
    pout1 = nc.alloc_psum_tensor("pout1", [128, NCOLS], F32).ap()

    DS = nc.alloc_semaphore("DS")  # coef dma completion (+16)
    S0 = nc.alloc_semaphore("S0")  # scratch memset done
    EVA = nc.alloc_semaphore("EVA")  # ACT's own o0 evac done
    J = nc.alloc_semaphore("J")  # j2y, j2x squares done
    G = nc.alloc_semaphore("G")  # iotas done (gyi, gxi)
    Q = nc.alloc_semaphore("Q")  # y squares done (intra-ACT RAW)
    Ax = nc.alloc_semaphore("Ax")  # x args ready
    E = nc.alloc_semaphore("E")  # exps ready (y0, y1, x0, x1)
    P = nc.alloc_semaphore("P")  # pout banks done (pout0, pout1)
    EV = nc.alloc_semaphore("EV")  # evac chunks done (o0a, o0b, o1a, o1b)
    DD = nc.alloc_semaphore("DD")  # output dma completions

    H = NCOLS // 2  # evac half width

    # ---- Sync: coef DMA in; second output store (o1 = rows 128:256) ----
    nc.sync.dma_start(coef, coef_d).then_inc(DS, 16)
    nc.sync.wait_ge(EV, 1)  # DVE's o1 evac complete
    nc.sync.dma_start(out[128:256, :], o1).then_inc(DD, 16)

    # ---- Pool: int16 grids ----
    nc.gpsimd.iota(gyi, [[1, MROWS]], base=0, channel_multiplier=0).then_inc(G, 1)
    nc.gpsimd.iota(gxi, [[1, NCOLS]], base=0, channel_multiplier=0).then_inc(G, 1)

    # ---- DVE: x args only (y side runs on ACT via Square), then evac ----
    nc.vector.memset(scr, 0.0).then_inc(S0, 1)
    nc.vector.wait_ge(G, 2)  # gxi iota done
    nc.vector.tensor_tensor(j2x, gxi, gxi, MULT).then_inc(J, 1)
    nc.vector.wait_ge(DS, 16)  # coef (argx scalar APs)
    nc.vector.wait_ge(J, 1)  # relaxed ordering: same-engine RAW needs a sem
    nc.vector.scalar_tensor_tensor(
        argx0, gxi, coef[:, 0:1], j2x, MULT, SUB
    ).then_inc(Ax, 1)
    nc.vector.scalar_tensor_tensor(
        argx1, gxi, coef[:, 1:2], j2x, MULT, SUB
    ).then_inc(Ax, 1)
    nc.vector.wait_ge(P, 2)
    nc.vector.tensor_copy(o1, pout1).then_inc(EV, 1)

    # ---- ACT: exps (table load auto-inserted first), evac left halves ----
    nc.scalar.wait_ge(S0, 1)
    nc.scalar.activation(scr[:, 1:2], scr[:, 0:1], EXP, bias=scr[:, 0:1])  # table-load trigger
    nc.scalar.wait_ge(G, 1)  # gyi iota done
    nc.scalar.wait_ge(DS, 16)  # coef (bias APs)
    nc.scalar.activation(sqy0, gyi, SQUARE, bias=coef[:, 2:3], scale=SQSC
                         ).then_inc(Q, 1)
    nc.scalar.wait_ge(Q, 1)  # own square complete (relaxed ordering)
    nc.scalar.activation(ey0, sqy0, EXP, bias=coef[:, 4:5], scale=-1.0).then_inc(E, 1)
    nc.scalar.activation(sqy1, gyi, SQUARE, bias=coef[:, 3:4], scale=SQSC
                         ).then_inc(Q, 1)
    nc.scalar.wait_ge(Q, 2)
    nc.scalar.activation(ey1, sqy1, EXP, bias=coef[:, 4:5], scale=-1.0).then_inc(E, 1)
    nc.scalar.wait_ge(Ax, 1)
    nc.scalar.activation(ex0, argx0, EXP, bias=coef[:, 6:7], scale=SC).then_inc(E, 1)
    nc.scalar.wait_ge(Ax, 2)
    nc.scalar.activation(ex1, argx1, EXP, bias=coef[:, 7:8], scale=SC).then_inc(E, 1)
    nc.scalar.wait_ge(P, 1)
    nc.scalar.copy(o0, pout0).then_inc(EVA, 1)
    nc.scalar.wait_ge(EVA, 1)  # own o0 copy complete (relaxed ordering)
    nc.scalar.dma_start(out[0:128, :], o0).then_inc(DD, 16)

    # ---- PE: k0 pair then k1 pair ----
    nc.tensor.wait_ge(E, 3)  # ey0, ey1, ex0
    nc.tensor.matmul(pout0, ey0[:, 0:128], ex0, start=True, stop=False,
                     skip_group_check=True)
    nc.tensor.matmul(pout1, ey0[:, 128:256], ex0, start=True, stop=False,
                     skip_group_check=True)
    nc.tensor.wait_ge(E, 4)  # ex1
    nc.tensor.matmul(pout0, ey1[:, 0:128], ex1, start=False, stop=True,
                     skip_group_check=True).then_inc(P, 1)
    nc.tensor.matmul(pout1, ey1[:, 128:256], ex1, start=False, stop=True,
                     skip_group_check=True).then_inc(P, 1)

    nc.compile()
    return nc


# revision 21
# speedup vs baseline: 1.2726x; 1.0030x over previous
"""Bezier Gaussian-splat raster kernel for 8 Trainium2 NeuronCores.

Reference computation (RES=1024, STEPS=256, SIGMA=0.01):
    curve = bezier(control_points)                # (2, 256)
    Ex[a,s] = exp(-(g[a]-x[s])^2 / (2 sigma^2))   # (1024, 256)
    Ey[b,s] = exp(-(g[b]-y[s])^2 / (2 sigma^2))
    OUT     = (Ey @ Ex^T) / 256                   # (1024, 1024) == raster.T

Sharding: 4 row-blocks x 2 col-blocks = 8 cores. Core i handles output rows
[256*(i//2), +256) and cols [512*(i%2), +512).

The curve is tiny (256 points, per the sharding hint), so the host
precomputes per-point exponent coefficients (one [128,8] f32 DMA per core —
big input DMAs are a loss here: ~700ns issue cost per dma_start plus slow
per-queue transfer). Everything O(res*steps) and O(res^2*steps) runs on
device:

  arg_x[s,j] = -c/RES^2 (j - X_s)^2            (X = RES*x', block-local)
             = (c/RES^2)(2X_s j - j^2) + bias_s,   bias_s = -(c/RES^2) X_s^2
  Pool iotas the int16 grids; DVE squares them and runs one
  scalar_tensor_tensor per (axis, k-chunk): t = (2X_s)*j - j^2. The ACT exp
  applies scale=c/RES^2 and the per-point bias AP in the same instruction
  (y side biases also carry -ln STEPS).

Raw Bass (no TileContext), hand-rolled semaphores. Rationale: the NEFF has a
fixed per-engine 51-semaphore-clear epilogue (~7us serial on the PE
sequencer) that begins once ALL engines retire their last instruction, so
the measured time is (body span) + ~8.4us. The body is kept minimal and the
two output stores are fire-and-forget: their ~1.5us DMA rides under the
epilogue (verified correct - the NEFF completion protocol waits for DMA).

Per-engine bodies:
  Sync : dma coef                              ... wait -> dma(out rows 128:)
  Pool : iota gyi [128,256], iota gxi [128,512]
  DVE  : j2y, argy0, argy1, j2x, argx0, argx1  ... evac halves of pout0/1
  ACT  : [exp-table load] exp y0,y1,x0,x1      ... evac halves, dma(out 0:128)
  PE   : k0-pair then k1-pair matmuls into 2 PSUM banks
"""

import math

import numpy as np

import concourse.bacc as bacc
import concourse.bass as bass
import concourse.mybir as mybir
from concourse.bass_utils import run_bass_kernel_spmd

RES = 1024
STEPS = 256
SIGMA = 0.01
INV2S2 = 1.0 / (2.0 * SIGMA * SIGMA)  # 5000.0
SC = INV2S2 / (RES * RES)  # exp scale: c / RES^2
SQSC = math.sqrt(SC)
LN_S = math.log(STEPS)

R_BLK = 4
C_BLK = 2
MROWS = RES // R_BLK  # 256
NCOLS = RES // C_BLK  # 512
N_CORES = 8

F32 = mybir.dt.float32
F16 = mybir.dt.float16
F8 = mybir.dt.float8e4
I16 = mybir.dt.int16

_CACHE: dict = {}

MULT = mybir.AluOpType.mult
SUB = mybir.AluOpType.subtract
EXP = mybir.ActivationFunctionType.Exp
SQUARE = mybir.ActivationFunctionType.Square


def _build_nc() -> bass.Bass:
    # Skip the ~3us all-engine EVSEM barrier Bass.__init__ emits after its
    # const-AP memsets; our first const-AP use is us later.
    _orig_barrier = bass.Bass.all_engine_barrier
    bass.Bass.all_engine_barrier = lambda self, **kw: None
    try:
        nc = bacc.Bacc(
            "TRN2",
            target_bir_lowering=False,
            debug=False,
            enable_asserts=False,
            enable_partition_id=False,
        )
    finally:
        bass.Bass.all_engine_barrier = _orig_barrier

    # Input (partition p = s mod 128, k-chunk = s div 128):
    #   coef cols: 2X_k0, 2X_k1, 2Y_k0, 2Y_k1, bcx_k0, bcx_k1, bcy_k0, bcy_k1
    coef_d = nc.dram_tensor("coef", [128, 8], F32, kind="ExternalInput").ap()
    out = nc.dram_tensor("out", [MROWS, NCOLS], F32, kind="ExternalOutput").ap()

    # SBUF
    coef = nc.alloc_sbuf_tensor("coef_sb", [128, 8], F32).ap()
    scr = nc.alloc_sbuf_tensor("scr", [128, 2], F32).ap()
    gxi = nc.alloc_sbuf_tensor("gxi_sb", [128, NCOLS], I16).ap()
    gyi = nc.alloc_sbuf_tensor("gyi_sb", [128, MROWS], I16).ap()
    j2x = nc.alloc_sbuf_tensor("j2x", [128, NCOLS], F32).ap()
    ex0 = nc.alloc_sbuf_tensor("ex0", [128, NCOLS], F16).ap()
    ex1 = nc.alloc_sbuf_tensor("ex1", [128, NCOLS], F16).ap()
    ey0 = nc.alloc_sbuf_tensor("ey0", [128, MROWS], F16).ap()
    ey1 = nc.alloc_sbuf_tensor("ey1", [128, MROWS], F16).ap()
    o0 = nc.alloc_sbuf_tensor("o0", [128, NCOLS], F32).ap()
    o1 = nc.alloc_sbuf_tensor("o1", [128, NCOLS], F32).ap()

    # PSUM: 4 arg banks + 2 matmul-output banks
    argx0 = nc.alloc_psum_tensor("argx0", [128, NCOLS], F32).ap()
    argx1 = nc.alloc_psum_tensor("argx1", [128, NCOLS], F32).ap()
    sqy0 = nc.alloc_psum_tensor("sqy0", [128, MROWS], F32).ap()
    sqy1 = nc.alloc_psum_tensor("sqy1", [128, MROWS], F32).ap()
    pout0 = nc.alloc_psum_tensor("pout0", [128, NCOLS], F32).ap()
    pout1 = nc.alloc_psum_tensor("pout1", [128, NCOLS], F32).ap()

    DS = nc.alloc_semaphore("DS")  # coef dma completion (+16)
    S0 = nc.alloc_semaphore("S0")  # scratch memset done
    EVA = nc.alloc_semaphore("EVA")  # ACT's own o0 evac done
    J = nc.alloc_semaphore("J")  # j2y, j2x squares done
    G = nc.alloc_semaphore("G")  # iotas done (gyi, gxi)
    Q = nc.alloc_semaphore("Q")  # y squares done (intra-ACT RAW)
    Ax = nc.alloc_semaphore("Ax")  # x args ready
    E = nc.alloc_semaphore("E")  # exps ready (y0, y1, x0, x1)
    P = nc.alloc_semaphore("P")  # pout banks done (pout0, pout1)
    EV = nc.alloc_semaphore("EV")  # evac chunks done (o0a, o0b, o1a, o1b)
    DD = nc.alloc_semaphore("DD")  # output dma completions

    H = NCOLS // 2  # evac half width

    # ---- Sync: coef DMA in; second output store (o1 = rows 128:256) ----
    nc.sync.dma_start(coef, coef_d).then_inc(DS, 16)
    nc.sync.wait_ge(EV, 1)  # DVE's o1 evac complete
    nc.sync.dma_start(out[128:256, :], o1).then_inc(DD, 16)

    # ---- Pool: int16 grids ----
    nc.gpsimd.iota(gyi, [[1, MROWS]], base=0, channel_multiplier=0).then_inc(G, 1)
    nc.gpsimd.iota(gxi, [[1, NCOLS]], base=0, channel_multiplier=0).then_inc(G, 1)

    # ---- DVE: x args only (y side runs on ACT via Square), then evac ----
    nc.vector.memset(scr, 0.0).then_inc(S0, 1)
    nc.vector.wait_ge(G, 2)  # gxi iota done
    nc.vector.tensor_tensor(j2x, gxi, gxi, MULT).then_inc(J, 1)
    nc.vector.wait_ge(DS, 16)  # coef (argx scalar APs)
    nc.vector.wait_ge(J, 1)  # relaxed ordering: same-engine RAW needs a sem
    nc.vector.scalar_tensor_tensor(
        argx0, gxi, coef[:, 0:1], j2x, MULT, SUB
    ).then_inc(Ax, 1)
    nc.vector.scalar_tensor_tensor(
        argx1, gxi, coef[:, 1:2], j2x, MULT, SUB
    ).then_inc(Ax, 1)
    nc.vector.wait_ge(P, 2)
    nc.vector.tensor_copy(o1, pout1).then_inc(EV, 1)

    # ---- ACT: exps (table load auto-inserted first), evac left halves ----
    nc.scalar.wait_ge(S0, 1)
    nc.scalar.activation(scr[:, 1:2], scr[:, 0:1], EXP, bias=scr[:, 0:1])  # table-load trigger
    nc.scalar.wait_ge(G, 1)  # gyi iota done
    nc.scalar.wait_ge(DS, 16)  # coef (bias APs)
    nc.scalar.activation(sqy0, gyi, SQUARE, bias=coef[:, 2:3], scale=SQSC
                         ).then_inc(Q, 1)
    nc.scalar.wait_ge(Q, 1)  # own square complete (relaxed ordering)
    nc.scalar.activation(ey0, sqy0, EXP, bias=coef[:, 4:5], scale=-1.0).then_inc(E, 1)
    nc.scalar.activation(sqy1, gyi, SQUARE, bias=coef[:, 3:4], scale=SQSC
                         ).then_inc(Q, 1)
    nc.scalar.wait_ge(Ax, 1)
    nc.scalar.activation(ex0, argx0, EXP, bias=coef[:, 6:7], scale=SC).then_inc(E, 1)
    nc.scalar.wait_ge(Q, 2)
    nc.scalar.activation(ey1, sqy1, EXP, bias=coef[:, 4:5], scale=-1.0).then_inc(E, 1)
    nc.scalar.wait_ge(Ax, 2)
    nc.scalar.activation(ex1, argx1, EXP, bias=coef[:, 7:8], scale=SC).then_inc(E, 1)
    nc.scalar.wait_ge(P, 1)
    nc.scalar.copy(o0, pout0).then_inc(EVA, 1)
    nc.scalar.wait_ge(EVA, 1)  # own o0 copy complete (relaxed ordering)
    nc.scalar.dma_start(out[0:128, :], o0).then_inc(DD, 16)

    # ---- PE: k0 pair then k1 pair ----
    # E order: ey0, ex0, ey1, ex1 -- the k0 pair only needs ey0+ex0
    nc.tensor.wait_ge(E, 2)  # ey0, ex0
    nc.tensor.matmul(pout0, ey0[:, 0:128], ex0, start=True, stop=False,
                     skip_group_check=True)
    nc.tensor.matmul(pout1, ey0[:, 128:256], ex0, start=True, stop=False,
                     skip_group_check=True)
    nc.tensor.wait_ge(E, 4)  # ex1
    nc.tensor.matmul(pout0, ey1[:, 0:128], ex1, start=False, stop=True,
                     skip_group_check=True).then_inc(P, 1)
    nc.tensor.matmul(pout1, ey1[:, 128:256], ex1, start=False, stop=True,
                     skip_group_check=True).then_inc(P, 1)

    nc.compile()
    return nc
